# revision 6
# baseline (speedup 1.0000x reference)
"""MoE FFN (8 experts, top-2, SwiGLU) on 8 TRN2 NeuronCores.

Strategy (expert-parallel, per sharding hint):
  - Router runs on host (it is ~0.1% of the FLOPs; the hint says
    "replicate router"), in float64 => ground-truth top-2 selection.
  - Host dispatch (the "all-to-all"): tokens are gathered per expert,
    padded to a fixed capacity CAP, and core e computes expert e's
    SwiGLU FFN for its tokens, pre-scaled by the combine weight.
  - Host combine: scatter-add per-expert outputs back to token order.

Device kernel (per core): y = cw * (silu(x@w1.T) * (x@w3.T)) @ w2.T
  computed as xT [D, CAP] streamed in token blocks of 512:
    phase A: aT/bT [H, TN] = w1T/w3T.T @ xT   (PE, fp32r, PSUM accum over D)
             hT = silu(aT) * bT               (ACT silu + DVE mul -> fp32r)
    phase B: y [TN, D] = hT.T @ w2T           (PE, fp32r, accum over H)
             y *= cw (ACT Copy with per-partition scale), DMA out.
"""

import os
import sys

sys.path.insert(0, '/opt/pypackages')
sys.path.insert(0, '/opt/trn_rl_repo')

import numpy as np

NUM_EXPERTS = 8
TOP_K = 2
AUX_COEF = 0.01
B, S, D, H = 4, 2048, 1024, 684
T = B * S                      # 8192 tokens
CAP = 2304                     # per-expert token capacity (18 * 128); max seed-0 load is 2175
N_CORES = 8

_cache = {}


def _token_blocks():
    """Token-column blocks of the capacity: 4x512 + 1x256."""
    blocks = []
    off = 0
    while off < CAP:
        tn = min(512, CAP - off)
        blocks.append((off, tn))
        off += tn
    return blocks


def _h_tiles():
    tiles = []
    off = 0
    while off < H:
        hs = min(128, H - off)
        tiles.append((off, hs))
        off += hs
    return tiles


def _build(repeats=1, internal_io=False):
    import concourse.bass as bass
    import concourse.mybir as mybir
    from concourse import bacc
    from concourse.tile import TileContext

    f32 = mybir.dt.float32
    f32r = mybir.dt.float32r
    ACT = mybir.ActivationFunctionType

    nc = bacc.Bacc()
    if internal_io:
        # timing variant: tiny external I/O, real tensors in internal DRAM
        DUM = nc.declare_dram_parameter("DUM", [128, 16], f32, isOutput=False)
        DOUT = nc.declare_dram_parameter("DOUT", [128, 16], f32, isOutput=True)
        XT = nc.dram_tensor("XT", [D, CAP], f32)
        W1T = nc.dram_tensor("W1T", [D, H], f32)
        W3T = nc.dram_tensor("W3T", [D, H], f32)
        W2T = nc.dram_tensor("W2T", [H, D], f32)
        CW = nc.dram_tensor("CW", [128, CAP // 128], f32)
        Y = nc.dram_tensor("Y", [CAP, D], f32)
    else:
        XT = nc.declare_dram_parameter("XT", [D, CAP], f32, isOutput=False)
        W1T = nc.declare_dram_parameter("W1T", [D, H], f32, isOutput=False)
        W3T = nc.declare_dram_parameter("W3T", [D, H], f32, isOutput=False)
        W2T = nc.declare_dram_parameter("W2T", [H, D], f32, isOutput=False)
        CW = nc.declare_dram_parameter("CW", [128, CAP // 128], f32, isOutput=False)
        Y = nc.declare_dram_parameter("Y", [CAP, D], f32, isOutput=True)

    KC = D // 128              # 8 contraction chunks for phase A
    htiles = _h_tiles()        # 6 tiles (5x128 + 44) for H
    tblocks = _token_blocks()

    with TileContext(nc) as tc:
        with tc.tile_pool(name="wp", bufs=1) as wp, \
             tc.tile_pool(name="xp", bufs=2) as xp, \
             tc.tile_pool(name="hp", bufs=2) as hp, \
             tc.tile_pool(name="op", bufs=3) as op, \
             tc.tile_pool(name="psa", bufs=2, space="PSUM") as psa, \
             tc.tile_pool(name="psy", bufs=2, space="PSUM") as psy:

          for _rep in range(repeats):
            # --- weights: DMA directly into f32r tiles (HW rounds on read) ---
            w1r, w3r = [], []
            for k in range(KC):
                for (W, lst, nm) in ((W1T, w1r, "w1"), (W3T, w3r, "w3")):
                    wr = wp.tile([128, H], f32r, tag=f"{nm}r{k}")
                    nc.sync.dma_start(
                        out=wr[:], in_=W[k * 128:(k + 1) * 128, :].bitcast(f32r))
                    lst.append(wr)
            w2r = []
            for j, (hoff, hs) in enumerate(htiles):
                wr = wp.tile([128, D], f32r, tag=f"w2r{j}")
                nc.sync.dma_start(
                    out=wr[:hs, :], in_=W2T[hoff:hoff + hs, :].bitcast(f32r))
                w2r.append(wr)

            # combine weights, one column per 128-token sub-block
            cwt = wp.tile([128, CAP // 128], f32, tag="cw")
            nc.sync.dma_start(out=cwt[:], in_=CW[:])

            # --- token blocks ---
            for bi, (toff, tn) in enumerate(tblocks):
                # load x chunks for this block straight into f32r tiles
                xr = []
                for k in range(KC):
                    xrk = xp.tile([128, 512], f32r, tag=f"xr{k}")
                    nc.sync.dma_start(
                        out=xrk[:, :tn],
                        in_=XT[k * 128:(k + 1) * 128, toff:toff + tn].bitcast(f32r))
                    xr.append(xrk)

                # phase A: hT tiles [hs, tn]
                hts = []
                for j, (hoff, hs) in enumerate(htiles):
                    pa = psa.tile([128, 512], f32, tag="pa")
                    pb = psa.tile([128, 512], f32, tag="pb")
                    for k in range(KC):
                        nc.tensor.matmul(
                            pa[:hs, :tn], w1r[k][:, hoff:hoff + hs], xr[k][:, :tn],
                            start=(k == 0), stop=(k == KC - 1))
                    for k in range(KC):
                        nc.tensor.matmul(
                            pb[:hs, :tn], w3r[k][:, hoff:hoff + hs], xr[k][:, :tn],
                            start=(k == 0), stop=(k == KC - 1))
                    sa = hp.tile([128, 512], f32, tag="sa")
                    nc.scalar.activation(sa[:hs, :tn], pa[:hs, :tn], ACT.Silu)
                    ht = hp.tile([128, 512], f32r, tag=f"ht{j}")
                    nc.vector.tensor_mul(ht[:hs, :tn], sa[:hs, :tn], pb[:hs, :tn])
                    hts.append(ht)

                # phase B: y [tn, D] in 128-token sub-blocks
                for t in range(tn // 128):
                    tcol = (toff + t * 128) // 128       # cw column index
                    for n in range(D // 512):
                        py = psy.tile([128, 512], f32, tag="py")
                        for j, (hoff, hs) in enumerate(htiles):
                            nc.tensor.matmul(
                                py[:], hts[j][:hs, t * 128:(t + 1) * 128],
                                w2r[j][:hs, n * 512:(n + 1) * 512],
                                start=(j == 0), stop=(j == len(htiles) - 1))
                        yt = op.tile([128, 512], f32, tag="yt")
                        nc.scalar.activation(yt[:], py[:], ACT.Copy,
                                             scale=cwt[:, tcol:tcol + 1])
                        nc.sync.dma_start(
                            out=Y[toff + t * 128:toff + (t + 1) * 128,
                                  n * 512:(n + 1) * 512],
                            in_=yt[:])

          if internal_io:
            dt_ = wp.tile([128, 16], f32, tag="dum")
            nc.sync.dma_start(out=dt_[:], in_=DUM[:])
            nc.sync.dma_start(out=DOUT[:], in_=dt_[:])

    nc.compile()
    return nc


def _get_nc():
    if 'nc' not in _cache:
        _cache['nc'] = _build()
    return _cache['nc']


def _route(x, router_w):
    """Host router in float64. Returns top2 idx [T,2], probs [T,E] f64."""
    xf = x.reshape(-1, D).astype(np.float64)
    logits = xf @ router_w.astype(np.float64).T
    m = logits.max(axis=1, keepdims=True)
    e = np.exp(logits - m)
    probs = e / e.sum(axis=1, keepdims=True)
    top2 = np.argsort(-logits, axis=1, kind='stable')[:, :TOP_K]
    return top2, probs


def kernel(x, router_w, w1, w2, w3, _trace=False):
    x = np.ascontiguousarray(np.asarray(x, dtype=np.float32))
    router_w = np.asarray(router_w, dtype=np.float32)
    w1 = np.asarray(w1, dtype=np.float32)
    w2 = np.asarray(w2, dtype=np.float32)
    w3 = np.asarray(w3, dtype=np.float32)

    top2, probs = _route(x, router_w)
    xf = x.reshape(-1, D)

    # aux loss (host, f64 -> f32)
    density = np.bincount(top2[:, 0], minlength=NUM_EXPERTS) / float(T)
    aux = np.float32(AUX_COEF * np.sum(density * probs.mean(axis=0)) * NUM_EXPERTS)

    # dispatch
    in_maps = []
    expert_tokens = []
    overflow = []  # (expert, token_idx) computed on host as a safety valve
    for e in range(NUM_EXPERTS):
        te = np.nonzero((top2 == e).any(axis=1))[0]
        if len(te) > CAP:
            overflow.extend((e, int(t)) for t in te[CAP:])
            te = te[:CAP]
        expert_tokens.append(te)
        xt = np.zeros((D, CAP), dtype=np.float32)
        xt[:, :len(te)] = xf[te].T
        cw = np.zeros(CAP, dtype=np.float32)
        cw[:len(te)] = probs[te, e].astype(np.float32)
        in_maps.append({
            "XT": xt,
            "W1T": np.ascontiguousarray(w1[e].T),
            "W3T": np.ascontiguousarray(w3[e].T),
            "W2T": np.ascontiguousarray(w2[e].T),
            "CW": np.ascontiguousarray(cw.reshape(CAP // 128, 128).T),
        })

    from concourse.bass_utils import run_bass_kernel_spmd
    nc = _get_nc()
    res = run_bass_kernel_spmd(nc, in_maps, list(range(N_CORES)), trace=_trace)

    out = np.zeros((T, D), dtype=np.float32)
    for e in range(NUM_EXPERTS):
        te = expert_tokens[e]
        out[te] += res.results[e]["Y"][:len(te)]

    # host fallback for capacity overflow (never hit for the seed-0 input)
    for e, t in overflow:
        def silu(v):
            return v / (1.0 + np.exp(-v))
        h = silu(xf[t] @ w1[e].T) * (xf[t] @ w3[e].T)
        out[t] += np.float32(probs[t, e]) * (h @ w2[e].T)

    if _trace:
        _cache['last_results'] = res
    return out.reshape(B, S, D), aux


# revision 11
# speedup vs baseline: 1.2794x; 1.2794x over previous
"""MoE FFN (8 experts, top-2, SwiGLU) on 8 TRN2 NeuronCores.

Strategy (expert-parallel, per sharding hint):
  - Router runs on host (it is ~0.1% of the FLOPs; the hint says
    "replicate router"), in float64 => ground-truth top-2 selection.
  - Host dispatch (the "all-to-all"): tokens are gathered per expert,
    padded to a fixed capacity CAP, and core e computes expert e's
    SwiGLU FFN for its tokens, pre-scaled by the combine weight.
  - Host combine: scatter-add per-expert outputs back to token order.

Device kernel (per core): y = cw * (silu(x@w1.T) * (x@w3.T)) @ w2.T
  fp32r matmuls (full PE rate), H zero-padded to 768 so every tile is
  a uniform 128 partitions. xT [D, CAP] streams in token blocks of 512:
    phase A: aT/bT [HP, TN] = w1T/w3T.T @ xT  (PE, PSUM accum over D)
             hT = silu(aT) * bT               (ACT silu + DVE mul -> fp32r)
    phase B: y [TN, D] = hT.T @ w2T           (PE, accum over HP)
             y *= cw (ACT Copy with per-partition scale), DMA out.
"""

import os
import sys

sys.path.insert(0, '/opt/pypackages')
sys.path.insert(0, '/opt/trn_rl_repo')

import numpy as np

NUM_EXPERTS = 8
TOP_K = 2
AUX_COEF = 0.01
B, S, D, H = 4, 2048, 1024, 684
HP = 768                       # H padded to a multiple of 128
T = B * S                      # 8192 tokens
CAP = 2304                     # per-expert token capacity (18 * 128); max seed-0 load is 2175
N_CORES = 8

_cache = {}


def _token_blocks():
    blocks = []
    off = 0
    while off < CAP:
        tn = min(512, CAP - off)
        blocks.append((off, tn))
        off += tn
    return blocks


def _build(repeats=1, internal_io=False):
    import concourse.bass as bass
    import concourse.mybir as mybir
    from concourse import bacc
    from concourse.tile import TileContext

    f32 = mybir.dt.float32
    f32r = mybir.dt.float32r
    ACT = mybir.ActivationFunctionType

    nc = bacc.Bacc()
    if internal_io:
        DUM = nc.declare_dram_parameter("DUM", [128, 16], f32, isOutput=False)
        DOUT = nc.declare_dram_parameter("DOUT", [128, 16], f32, isOutput=True)
        XT = nc.dram_tensor("XT", [D, CAP], f32)
        W1T = nc.dram_tensor("W1T", [D, HP], f32)
        W3T = nc.dram_tensor("W3T", [D, HP], f32)
        W2T = nc.dram_tensor("W2T", [HP, D], f32)
        CW = nc.dram_tensor("CW", [128, CAP // 128], f32)
        Y = nc.dram_tensor("Y", [CAP, D], f32)
    else:
        XT = nc.declare_dram_parameter("XT", [D, CAP], f32, isOutput=False)
        W1T = nc.declare_dram_parameter("W1T", [D, HP], f32, isOutput=False)
        W3T = nc.declare_dram_parameter("W3T", [D, HP], f32, isOutput=False)
        W2T = nc.declare_dram_parameter("W2T", [HP, D], f32, isOutput=False)
        CW = nc.declare_dram_parameter("CW", [128, CAP // 128], f32, isOutput=False)
        Y = nc.declare_dram_parameter("Y", [CAP, D], f32, isOutput=True)

    KC = D // 128              # 8 contraction chunks for phase A
    JC = HP // 128             # 6 h tiles
    tblocks = _token_blocks()

    # 3D views for single-DMA loads
    XTv = XT.rearrange("(c p) n -> p c n", p=128)        # [128, KC, CAP]
    W1v = W1T.rearrange("(c p) h -> p c h", p=128)       # [128, KC, HP]
    W3v = W3T.rearrange("(c p) h -> p c h", p=128)
    W2v = W2T.rearrange("(j p) d -> p j d", p=128)       # [128, JC, D]


    with TileContext(nc) as tc:
        with tc.tile_pool(name="wp", bufs=1) as wp, \
             tc.tile_pool(name="xp", bufs=2) as xp, \
             tc.tile_pool(name="hp", bufs=2) as hp, \
             tc.tile_pool(name="op", bufs=3) as op, \
             tc.tile_pool(name="psa", bufs=1, space="PSUM") as psa, \
             tc.tile_pool(name="psy", bufs=2, space="PSUM") as psy:

          for _rep in range(repeats):
            def load_x(toff, tn, per_k):
                xrt = xp.tile([128, KC, 512], f32r, tag="xr")
                if per_k:
                    return xrt, (toff, tn)   # DMAs emitted interleaved below
                nc.sync.dma_start(
                    out=xrt[:, :, :tn],
                    in_=XTv[:, :, toff:toff + tn].bitcast(f32r))
                return xrt, None

            # weight tiles; block-0 x / w1 / w3 stream per-k so the first
            # accumulation chunks start after ~1 MB instead of ~8 MB.
            w1r = wp.tile([128, KC, HP], f32r, tag="w1r")
            w3r = wp.tile([128, KC, HP], f32r, tag="w3r")
            w2r = wp.tile([128, JC, D], f32r, tag="w2r")
            xr_next, pend = load_x(*tblocks[0], per_k=True)
            toff0, tn0 = pend
            cwt = wp.tile([128, CAP // 128], f32, tag="cw")
            for k in range(KC):
                nc.sync.dma_start(
                    out=xr_next[:, k, :tn0],
                    in_=XTv[:, k, toff0:toff0 + tn0].bitcast(f32r))
                nc.sync.dma_start(out=w1r[:, k, :], in_=W1v[:, k, :].bitcast(f32r))
                nc.sync.dma_start(out=w3r[:, k, :], in_=W3v[:, k, :].bitcast(f32r))
                if k == 0:
                    nc.sync.dma_start(out=cwt[:], in_=CW[:])
            for j in range(JC):
                nc.sync.dma_start(out=w2r[:, j, :], in_=W2v[:, j, :].bitcast(f32r))

            JG = 3                       # j-group size (2*JG psum banks for phase A)
            for bi, (toff, tn) in enumerate(tblocks):
                xr = xr_next
                if bi + 1 < len(tblocks):
                    xr_next, _ = load_x(*tblocks[bi + 1], per_k=False)

                # phase A, k-outer within j-groups: PE starts on chunk k as
                # soon as its x/w slices land.
                hts = []
                for g0 in range(0, JC, JG):
                    js = range(g0, min(g0 + JG, JC))
                    pas = {j: psa.tile([128, 512], f32, tag=f"pa{j - g0}", name=f"pa_{bi}_{j}") for j in js}
                    pbs = {j: psa.tile([128, 512], f32, tag=f"pb{j - g0}", name=f"pb_{bi}_{j}") for j in js}
                    for k in range(KC):
                        for j in js:
                            nc.tensor.matmul(
                                pas[j][:, :tn], w1r[:, k, j * 128:(j + 1) * 128],
                                xr[:, k, :tn], start=(k == 0), stop=(k == KC - 1))
                            nc.tensor.matmul(
                                pbs[j][:, :tn], w3r[:, k, j * 128:(j + 1) * 128],
                                xr[:, k, :tn], start=(k == 0), stop=(k == KC - 1))
                    for j in js:
                        sa = hp.tile([128, 512], f32, tag="sa")
                        nc.scalar.activation(sa[:, :tn], pas[j][:, :tn], ACT.Silu)
                        ht = hp.tile([128, 512], f32r, tag=f"ht{j}")
                        nc.vector.tensor_mul(ht[:, :tn], sa[:, :tn], pbs[j][:, :tn])
                        hts.append(ht)

                # phase B: y [tn, D] in 128-token sub-blocks
                for t in range(tn // 128):
                    tcol = (toff + t * 128) // 128       # cw column index
                    yt = op.tile([128, D], f32, tag="yt")
                    for n in range(D // 512):
                        py = psy.tile([128, 512], f32, tag="py")
                        for j in range(JC):
                            nc.tensor.matmul(
                                py[:], hts[j][:, t * 128:(t + 1) * 128],
                                w2r[:, j, n * 512:(n + 1) * 512],
                                start=(j == 0), stop=(j == JC - 1))
                        nc.scalar.activation(yt[:, n * 512:(n + 1) * 512], py[:],
                                             ACT.Copy, scale=cwt[:, tcol:tcol + 1])
                        nc.scalar.dma_start(
                            out=Y[toff + t * 128:toff + (t + 1) * 128,
                                  n * 512:(n + 1) * 512],
                            in_=yt[:, n * 512:(n + 1) * 512])

          if internal_io:
            dt_ = wp.tile([128, 16], f32, tag="dum")
            nc.sync.dma_start(out=dt_[:], in_=DUM[:])
            nc.sync.dma_start(out=DOUT[:], in_=dt_[:])

    nc.compile()
    return nc


def _get_nc():
    if 'nc' not in _cache:
        _cache['nc'] = _build()
    return _cache['nc']


def _route(x, router_w):
    """Host router in float64. Returns top2 idx [T,2], probs [T,E] f64."""
    xf = x.reshape(-1, D).astype(np.float64)
    logits = xf @ router_w.astype(np.float64).T
    m = logits.max(axis=1, keepdims=True)
    e = np.exp(logits - m)
    probs = e / e.sum(axis=1, keepdims=True)
    top2 = np.argsort(-logits, axis=1, kind='stable')[:, :TOP_K]
    return top2, probs


def _make_in_maps(x, router_w, w1, w2, w3):
    top2, probs = _route(x, router_w)
    xf = x.reshape(-1, D)

    density = np.bincount(top2[:, 0], minlength=NUM_EXPERTS) / float(T)
    aux = np.float32(AUX_COEF * np.sum(density * probs.mean(axis=0)) * NUM_EXPERTS)

    in_maps, expert_tokens, overflow = [], [], []
    for e in range(NUM_EXPERTS):
        te = np.nonzero((top2 == e).any(axis=1))[0]
        if len(te) > CAP:
            overflow.extend((e, int(t)) for t in te[CAP:])
            te = te[:CAP]
        expert_tokens.append(te)
        xt = np.zeros((D, CAP), dtype=np.float32)
        xt[:, :len(te)] = xf[te].T
        cw = np.zeros(CAP, dtype=np.float32)
        cw[:len(te)] = probs[te, e].astype(np.float32)
        w1p = np.zeros((D, HP), dtype=np.float32); w1p[:, :H] = w1[e].T
        w3p = np.zeros((D, HP), dtype=np.float32); w3p[:, :H] = w3[e].T
        w2p = np.zeros((HP, D), dtype=np.float32); w2p[:H, :] = w2[e].T
        in_maps.append({
            "XT": xt, "W1T": w1p, "W3T": w3p, "W2T": w2p,
            "CW": np.ascontiguousarray(cw.reshape(CAP // 128, 128).T),
        })
    return in_maps, expert_tokens, overflow, probs, aux


def kernel(x, router_w, w1, w2, w3, _trace=False):
    x = np.ascontiguousarray(np.asarray(x, dtype=np.float32))
    router_w = np.asarray(router_w, dtype=np.float32)
    w1 = np.asarray(w1, dtype=np.float32)
    w2 = np.asarray(w2, dtype=np.float32)
    w3 = np.asarray(w3, dtype=np.float32)
    xf = x.reshape(-1, D)

    in_maps, expert_tokens, overflow, probs, aux = _make_in_maps(
        x, router_w, w1, w2, w3)

    from concourse.bass_utils import run_bass_kernel_spmd
    nc = _get_nc()
    res = run_bass_kernel_spmd(nc, in_maps, list(range(N_CORES)), trace=_trace)

    out = np.zeros((T, D), dtype=np.float32)
    for e in range(NUM_EXPERTS):
        te = expert_tokens[e]
        out[te] += res.results[e]["Y"][:len(te)]

    # host fallback for capacity overflow (never hit for the seed-0 input)
    for e, t in overflow:
        def silu(v):
            return v / (1.0 + np.exp(-v))
        h = silu(xf[t] @ w1[e].T) * (xf[t] @ w3[e].T)
        out[t] += np.float32(probs[t, e]) * (h @ w2[e].T)

    if _trace:
        _cache['last_results'] = res
    return out.reshape(B, S, D), aux
